# revision 7
# baseline (speedup 1.0000x reference)
"""Trainium2 Bass kernel for nn_Loss_Synonymy.

reference:
    diff = S1 - S2                       # [B, 256]
    d    = sqrt(sum(diff^2, axis=-1))    # [B]
    t    = tanh(d)
    err  = where(score >= 0.8, relu(1 - t), relu(1 + t))
    out  = sum(err) / B

Since tanh(d) in [0, 1] for d >= 0, both relu branches are pass-through:
    err = 1 + sign * tanh(d),  sign = -1 if score >= 0.8 else +1
    sum(err) = B + sum(sign * tanh(d))
so the device only accumulates sum(sign * tanh(d)); the host adds B and
divides.

Data-parallel over 8 NeuronCores: each core streams its 32768-row shard
(2 x 32 MiB) from HBM and emits a single f32 partial of
sum(sign * tanh(d)).

Per-core layout: tile t covers 2048 consecutive rows as [128 part, 4096
free] (partition p holds rows t*2048 + p*16 + j for j in 0..15: 16 KiB
contiguous per partition -> near-peak DMA efficiency). Row chunk (t, j)
is a [128, 256] free-dim reduction: VectorE computes the diff (bf16 out:
the square-sum tolerates 2^-9 relative error and 16-bit doubles DVE
throughput), then each tile's 16 chunk square-sums are split ScalarE
(prefix, Square + accum_out) / VectorE (suffix, scalar_tensor_tensor) so
neither engine exceeds the HBM streaming floor.

Row-chunk c's dist/score/sign/tanh column is c's accumulation slot:
ScalarE chunks occupy columns 0..NA-1 in issue order, VectorE chunks
NA..255, so every per-tile sqrt covers one contiguous column run and the
taper's sqrts batch into two instructions. The score DMAs are permuted
to match at kernel start.

Tail minimization (the naive epilogue idled DMA for ~21 us):
  - dist = sqrt(sumsq) is folded into the main loop per tile. A dummy
    Sqrt up front pins the sqrt_and_others ACT table set; Square is
    filler in every set, so no mid-loop ACT_TABLE_LOAD occurs and the
    per-tile Sqrts are load-free.
  - The last 2 MiB tile is tapered into j-range sub-tiles (8/4/2/1/1
    rows per partition) so the post-last-byte serial chain is one
    [128, 256] diff + one accum instead of a 4096-wide diff + 10
    squares.
  - One Tanh over the merged [128, 256] dist + a sign-weighted
    accumulate finish on-chip; only the single exp-set table load
    (~2.7 us) remains exposed. The sign tile {-2,0} is precomputed
    from the scores during streaming.
  - The [128, 1] per-partition partial is dotted with ones on TensorE
    so the output DMA is one 4-byte descriptor: a [128, 1] store
    shatters into 128 4 B descriptors whose completion semaphore
    lags ~7 us.
"""

import numpy as np

import concourse.bass as bass
import concourse.tile as tile
from concourse import bacc, mybir
from concourse.bass_utils import run_bass_kernel_spmd

F32 = mybir.dt.float32
BF16 = mybir.dt.bfloat16
AF = mybir.ActivationFunctionType
ALU = mybir.AluOpType

B = 262144
D = 256
NCORES = 8
BL = B // NCORES          # 32768 rows per core
J = 16                    # rows per partition per full tile
TILE_ROWS = 128 * J       # 2048
NT = BL // TILE_ROWS      # 16 tile slots (last one tapered)
FREE = J * D              # 4096 (2 MiB per [128, FREE] f32 tile)
KA = 8                    # full-tile chunks j < KA -> ScalarE, rest -> VectorE
THRESH = 0.8
BUFS_IN = 4               # buffering depth for input pools
BUFS_DIFF = 3
# (j0, j1, ka) sub-tiles of the last tile, finest last; ka = how many of
# the sub-tile's leading chunks ride ScalarE (VectorE takes the rest).
TAPER = [(0, 8, 4), (8, 12, 2), (12, 14, 1), (14, 15, 1), (15, 16, 0)]

_NC_CACHE = {}


def _chunk_cols():
    """Map chunk (t, j) -> (engine, dist column); ScalarE chunks fill
    columns 0..NA-1 in issue order, VectorE chunks NA..NT*J-1."""
    plan = [(t, 0, J, KA) for t in range(NT - 1)]
    plan += [(NT - 1, j0, j1, ka) for j0, j1, ka in TAPER]
    na = sum(min(ka, j1 - j0) for _, j0, j1, ka in plan)
    amap, dmap = {}, {}
    ca, cd = 0, na
    for t, j0, j1, ka in plan:
        for j in range(j0, j1):
            if j - j0 < ka:
                amap[(t, j)] = ca
                ca += 1
            else:
                dmap[(t, j)] = cd
                cd += 1
    return plan, na, amap, dmap


def _build_nc():
    nc = bacc.Bacc(
        "TRN2", target_bir_lowering=False, debug=False, num_devices=NCORES
    )

    s1 = nc.dram_tensor("s1", [BL, D], F32, kind="ExternalInput").ap()
    s2 = nc.dram_tensor("s2", [BL, D], F32, kind="ExternalInput").ap()
    score = nc.dram_tensor("score", [BL], F32, kind="ExternalInput").ap()
    partial = nc.dram_tensor("partial", [1, 1], F32, kind="ExternalOutput").ap()

    # [NT, 128, J, D] views: tile t / partition p / row-chunk j / feature d
    s1_r = s1.rearrange("(t p j) d -> t p j d", t=NT, p=128, j=J)
    s2_r = s2.rearrange("(t p j) d -> t p j d", t=NT, p=128, j=J)
    score_r = score.rearrange("(t p j) -> p t j", t=NT, p=128, j=J)

    plan, NA, amap, dmap = _chunk_cols()
    NC = NT * J  # 256 chunk columns

    # Discarded elementwise outputs (only accum_out matters). Raw sbuf
    # tensors (not pool tiles) so Tile's release machinery ignores them.
    scr_dve = nc.alloc_sbuf_tensor("scr_dve", [128, D], F32).ap()
    scr_bf = nc.alloc_sbuf_tensor("scr_bf", [128, D], BF16).ap()
    scr_act = nc.alloc_sbuf_tensor("scr_act", [128, D], F32).ap()

    with tile.TileContext(nc) as tc:
        with (
            tc.tile_pool(name="in1", bufs=BUFS_IN) as p_in1,
            tc.tile_pool(name="in2", bufs=BUFS_IN) as p_in2,
            tc.tile_pool(name="diff", bufs=BUFS_DIFF) as p_diff,
            tc.tile_pool(name="persist", bufs=1) as p_per,
            tc.tile_pool(name="psum", bufs=1, space=bass.MemorySpace.PSUM) as p_ps,
        ):
            sumsq = p_per.tile([128, NC], F32, tag="sumsq")
            dist = p_per.tile([128, NC], F32, tag="dist")
            score_sb = p_per.tile([128, NC], F32, tag="score_sb")
            sgn = p_per.tile([128, NC], F32, tag="sgn")
            th = p_per.tile([128, NC], F32, tag="th")
            part_sb = p_per.tile([128, 1], F32, tag="part_sb")
            ones_sb = p_per.tile([128, 1], F32, tag="ones_sb")
            out_sb = p_per.tile([1, 1], F32, tag="out_sb")
            psum = p_ps.tile([1, 1], F32, tag="psum")

            # Pin the sqrt_and_others ACT table set before any Square
            # (square is filler in every set) so the per-tile Sqrts
            # below never trigger a mid-loop table load.
            nc.vector.memset(scr_dve[:, 0:1], 0.0)
            nc.scalar.activation(scr_act[:, 0:1], scr_dve[:, 0:1], AF.Sqrt)
            nc.gpsimd.memset(ones_sb[:], 1.0)

            # Scores ride SWDGE once, permuted into chunk-column order:
            # contiguous (t, j) runs with a common column run become one
            # DMA (two big ones for the full tiles, tiny ones for the
            # taper splits).
            def score_dma(t0, t1, ja, jb, col):
                n = (t1 - t0) * (jb - ja)
                nc.gpsimd.dma_start(
                    score_sb[:, col : col + n].rearrange(
                        "p (t j) -> p t j", j=jb - ja
                    ),
                    score_r[:, t0:t1, ja:jb],
                )

            score_dma(0, NT - 1, 0, KA, amap[(0, 0)])
            score_dma(0, NT - 1, KA, J, dmap[(0, KA)])
            for j0, j1, ka in TAPER:
                kk = min(ka, j1 - j0)
                if kk > 0:
                    score_dma(NT - 1, NT, j0, j0 + kk, amap[(NT - 1, j0)])
                if kk < j1 - j0:
                    score_dma(NT - 1, NT, j0 + kk, j1, dmap[(NT - 1, j0 + kk)])

            nc.vector.tensor_scalar(
                sgn[:], score_sb[:], THRESH, -2.0, ALU.is_ge, ALU.mult
            )

            def do_tile(t, j0, j1, ka, sqrts=True):
                jj = j1 - j0
                a = p_in1.tile([128, jj * D], F32, tag="a")
                nc.sync.dma_start(
                    a[:].rearrange("p (j d) -> p j d", d=D), s1_r[t, :, j0:j1]
                )
                b = p_in2.tile([128, jj * D], F32, tag="b")
                nc.sync.dma_start(
                    b[:].rearrange("p (j d) -> p j d", d=D), s2_r[t, :, j0:j1]
                )
                diff = p_diff.tile([128, jj * D], BF16, tag="diff")
                nc.vector.tensor_sub(diff[:], a[:], b[:])

                for j in range(j0, j1):
                    chunk = diff[:, (j - j0) * D : (j - j0 + 1) * D]
                    if j - j0 < ka:
                        c = amap[(t, j)]
                        nc.scalar.activation(
                            scr_act,
                            chunk,
                            AF.Square,
                            accum_out=sumsq[:, c : c + 1],
                        )
                    else:
                        c = dmap[(t, j)]
                        nc.vector.scalar_tensor_tensor(
                            scr_bf,
                            chunk,
                            1.0,
                            chunk,
                            ALU.mult,
                            ALU.mult,
                            accum_out=sumsq[:, c : c + 1],
                        )
                if sqrts:
                    kk = min(ka, jj)
                    if kk > 0:
                        c = amap[(t, j0)]
                        nc.scalar.activation(
                            dist[:, c : c + kk], sumsq[:, c : c + kk], AF.Sqrt
                        )
                    if kk < jj:
                        c = dmap[(t, j0 + kk)]
                        nc.scalar.activation(
                            dist[:, c : c + jj - kk],
                            sumsq[:, c : c + jj - kk],
                            AF.Sqrt,
                        )

            for t in range(NT - 1):
                do_tile(t, 0, J, KA)
            # Taper: per-sub-tile sqrts are batched into one ScalarE-run
            # sqrt per engine region, emitted after that region's last
            # accumulate.
            ta0 = amap[(NT - 1, 0)]
            td0 = dmap[(NT - 1, TAPER[0][2])]
            na_taper = sum(min(ka, j1 - j0) for j0, j1, ka in TAPER)
            nd_taper = J - na_taper
            for i, (j0, j1, ka) in enumerate(TAPER):
                do_tile(NT - 1, j0, j1, ka, sqrts=False)
                if i == len(TAPER) - 2:
                    # last ScalarE chunk lives in TAPER[-2]
                    nc.scalar.activation(
                        dist[:, ta0 : ta0 + na_taper],
                        sumsq[:, ta0 : ta0 + na_taper],
                        AF.Sqrt,
                    )
            nc.scalar.activation(
                dist[:, td0 : td0 + nd_taper],
                sumsq[:, td0 : td0 + nd_taper],
                AF.Sqrt,
            )

            # Epilogue: one exp-set table load, then
            # part = sum_c (sgn + 1) * tanh(dist); dot with ones on
            # TensorE so the store is a single 4 B descriptor.
            nc.scalar.activation(th[:], dist[:], AF.Tanh)
            nc.vector.scalar_tensor_tensor(
                scr_dve,
                sgn[:],
                1.0,
                th[:],
                ALU.add,
                ALU.mult,
                accum_out=part_sb[:, 0:1],
            )
            nc.tensor.matmul(psum[:], part_sb[:], ones_sb[:])
            nc.vector.tensor_copy(out_sb[:], psum[:])
            nc.sync.dma_start(partial, out_sb[:])

    nc.compile()
    return nc


def _get_nc():
    if "nc" not in _NC_CACHE:
        _NC_CACHE["nc"] = _build_nc()
    return _NC_CACHE["nc"]


def make_in_maps(S1_out, S2_out, synonymy_score):
    in_maps = []
    for c in range(NCORES):
        lo, hi = c * BL, (c + 1) * BL
        in_maps.append(
            {
                "s1": np.ascontiguousarray(S1_out[lo:hi], dtype=np.float32),
                "s2": np.ascontiguousarray(S2_out[lo:hi], dtype=np.float32),
                "score": np.ascontiguousarray(
                    synonymy_score[lo:hi], dtype=np.float32
                ),
            }
        )
    return in_maps


def combine(results):
    total = np.float64(0.0)
    for r in results:
        total += np.float64(r["partial"].reshape(()))
    return np.asarray((B + total) / B, dtype=np.float32)


def run(S1_out, S2_out, synonymy_score, trace=False, **trace_kwargs):
    nc = _get_nc()
    in_maps = make_in_maps(S1_out, S2_out, synonymy_score)
    res = run_bass_kernel_spmd(
        nc, in_maps, list(range(NCORES)), trace=trace, **trace_kwargs
    )
    return combine(res.results), res


def kernel(S1_out, S2_out, synonymy_score):
    out, _ = run(S1_out, S2_out, synonymy_score)
    return out
